# revision 12
# baseline (speedup 1.0000x reference)
"""Trainium2 Bass kernel for nn_CCNLoss (v9: unit-granular pipeline).

loss = mean(|p - t|) + 0.5 * sum(arccos(clip(cos, -1+1e-7, 1-1e-7))) + |crm(p) - crm(t)|

where cos[h,w] = sum_c sab_c / sqrt(saa_c * sbb_c), s** = sum_b of pt/pp/tt.

Algebraic facts (validated numerically against the reference):
  * crm(img) = mean(softmax(X, 0)) == 1/m exactly -> the crm term is 0; dropped.
  * arccos(x) = 2*atan(sqrt((1-x)/(1+x))); the 2 cancels the 0.5 weight.
  * u' = min(cos, CLIP) in f32; t1 = 1-u' (exact by Sterbenz; == 1-CLIP when
    clipped), t2 = 1+u'; theta = 2*atan(t1 * rsqrt(t1*t2)).
  * fp16 inputs/products perturb the loss ~3e-5 relative (measured).

Per-core structure (h-slab of 128 rows on 128 partitions), tuned from
measured HW rates (DVE fp16 TT 0.56ns/elem 2x, DVE f32 TS 0.84ns 2x_2p,
ACT 0.88ns, Pool ~2.3ns, PE 0.42-0.83ns/col by pstate):
  * p and t are host-packed per (channel, chunk) unit: ONE 1MB DMA per
    unit (8KB contiguous per partition-row) -> few DGE configs, first
    unit lands ~9us, all data by ~22us.
  * Per unit: pt-mul (V), fused p/t squares (V or ACT per table), d (V),
    |d|+accum (ACT Abs with accum_out), 12 PE matmuls, rsqrt pair (ACT),
    inv (Pool), cos-mul (V).
  * Last unit runs in w-halves to shorten the serial tail cascade; the
    arctan needs one table swap (Abs/Square/Rsqrt live in one set,
    preloaded via a dummy op during the DMA fill).
"""

import numpy as np
from contextlib import ExitStack

import concourse.bass as bass
import concourse.bacc as bacc
import concourse.tile as tile
from concourse import mybir
from concourse.bass_utils import run_bass_kernel_spmd

B, C, H, W = 4, 3, 1024, 1024
NCORES = 8
HC = H // NCORES
P = 128
WC = 512
NCH = 2
NU = C * NCH              # 6 (channel, chunk) units

F32 = mybir.dt.float32
F16 = mybir.dt.float16
AF = mybir.ActivationFunctionType
OP = mybir.AluOpType
AX = mybir.AxisListType

CLIP_HI = float(np.float32(1.0 - 1e-7))
N_WARM = 14

UNITS = [(0, 0), (0, 1), (1, 0), (1, 1), (2, 0), (2, 1)]
SQ_ENG = ['s', 's', 'v', 'v', 's', 'v']   # per-unit square engine
_CACHE = {}


def _body(tc, inputs, identf16, res_out):
    nc = tc.nc
    with ExitStack() as ctx:
        inpool = ctx.enter_context(tc.tile_pool(name="inp", bufs=1))
        prodp = ctx.enter_context(tc.tile_pool(name="prod", bufs=3))
        dpool = ctx.enter_context(tc.tile_pool(name="dsc", bufs=2))
        scrp = ctx.enter_context(tc.tile_pool(name="scr", bufs=1))
        work = ctx.enter_context(tc.tile_pool(name="work", bufs=2))
        consts = ctx.enter_context(tc.tile_pool(name="consts", bufs=1))
        psum = ctx.enter_context(tc.tile_pool(name="ps", bufs=2, space="PSUM"))
        outp = ctx.enter_context(tc.tile_pool(name="outp", bufs=1))

        idw = consts.tile([P, P], F16)
        nc.sync.dma_start(out=idw, in_=identf16)

        # res: col u = sum|d| of unit u; col 7 = atan sum
        res = outp.tile([P, 8], F32)

        ptk = [inpool.tile([P, 2, B, WC], F16, name=f"ptk{u}", bufs=1)
               for u in range(NU)]
        for u, (c, k) in enumerate(UNITS):
            nc.sync.dma_start(out=ptk[u], in_=inputs[c, k])

        # force the Abs_reciprocal_sqrt table set before any Square lands
        wsrc = consts.tile([P, WC], F16)
        nc.gpsimd.memset(wsrc, 0.0)
        tdum = consts.tile([P, 1], F32)
        nc.scalar.activation(tdum, wsrc[:, 0:1], AF.Abs_reciprocal_sqrt)

        warm = psum.tile([P, WC], F32, tag="warm", bufs=1)
        for _ in range(N_WARM):
            nc.tensor.matmul(warm, idw, wsrc, start=True, stop=True)

        scr = scrp.tile([P, 2, B, WC], F16, name="scr")
        prod = {}
        dsc = {}   # one d-tile per unit PAIR: [P, 2, B, WC], halves by unit

        def products(u, ws=None):
            """pt mul + fused squares + d for unit u over w-slice."""
            if u not in prod:
                prod[u] = prodp.tile([P, 3, B, WC], F16, tag="prod",
                                     name=f"pr{u}", bufs=3)
            if u // 2 not in dsc:
                dsc[u // 2] = dpool.tile([P, 2, B, WC], F16, tag="dsc",
                                         name=f"d{u // 2}", bufs=2)
            w0, w1 = ws if ws else (0, WC)
            s = slice(w0, w1)
            pr = prod[u]
            pk = ptk[u][:, 0, :, s]
            tk = ptk[u][:, 1, :, s]
            nc.vector.tensor_mul(pr[:, 0, :, s], pk, tk)
            if SQ_ENG[u] == 's':
                nc.scalar.square(pr[:, 1:3, :, s], ptk[u][:, :, :, s])
            else:
                nc.vector.tensor_mul(pr[:, 1:3, :, s], ptk[u][:, :, :, s],
                                     ptk[u][:, :, :, s])
            nc.vector.tensor_sub(dsc[u // 2][:, u % 2, :, s], pk, tk)

        def absred(pair):
            nc.scalar.activation(
                scr, dsc[pair], AF.Abs, accum_out=res[:, pair:pair + 1]
            )

        cosq = {k: work.tile([P, C, WC], F16, tag=f"cosq{k}", bufs=1,
                             name=f"cosq{k}")
                for k in range(NCH)}

        def pe_unit(u, ws=None):
            w0, w1 = ws if ws else (0, WC)
            ps = psum.tile([P, 3, WC], F32, tag="ps", name=f"ps{u}{w0}")
            for q in range(3):
                for b in range(B):
                    nc.tensor.matmul(
                        ps[:, q, w0:w1], idw, prod[u][:, q, b, w0:w1],
                        start=(b == 0), stop=(b == B - 1),
                    )
            return ps

        def tail(u, ps, ws=None):
            c, k = UNITS[u]
            w0, w1 = ws if ws else (0, WC)
            wsl = slice(w0, w1)
            rinv = work.tile([P, 2, WC], F16, tag="rinv", name=f"ri{u}{w0}")
            nc.scalar.activation(rinv[:, :, wsl], ps[:, 1:3, wsl],
                                 AF.Abs_reciprocal_sqrt)
            inv = work.tile([P, WC], F16, tag="inv", name=f"iv{u}{w0}")
            nc.gpsimd.tensor_mul(inv[:, wsl], rinv[:, 0, wsl], rinv[:, 1, wsl])
            nc.vector.tensor_mul(cosq[k][:, c, wsl], ps[:, 0, wsl],
                                 inv[:, wsl])

        chn = {}
        ssb = outp.tile([P, NCH, WC], F32)

        def chain(k, ws=None, eng='v'):
            w0, w1 = ws if ws else (0, WC)
            s = slice(w0, w1)
            t = chn.setdefault(k, dict(
                cs=work.tile([P, WC], F16, tag="cs", bufs=1, name=f"cs{k}"),
                co=work.tile([P, WC], F16, tag="co", bufs=1, name=f"co{k}"),
                u1=work.tile([P, WC], F32, tag="u1", bufs=1, name=f"u1{k}"),
                t1=work.tile([P, WC], F32, tag="t1", bufs=1, name=f"t1{k}"),
                t2=work.tile([P, WC], F32, tag="t2", bufs=1, name=f"t2{k}"),
                mm=work.tile([P, WC], F32, tag="mm", bufs=1, name=f"mm{k}"),
                sr=work.tile([P, WC], F32, tag="sr", bufs=1, name=f"sr{k}"),
            ))
            cq = cosq[k]
            add_eng = nc.gpsimd if eng == 'g' else nc.vector
            add_eng.tensor_add(t["cs"][:, s], cq[:, 0, s], cq[:, 1, s])
            add_eng.tensor_add(t["co"][:, s], t["cs"][:, s], cq[:, 2, s])
            nc.vector.tensor_scalar(
                out=t["u1"][:, s], in0=t["co"][:, s], scalar1=CLIP_HI,
                scalar2=None, op0=OP.min,
            )
            nc.vector.tensor_scalar(
                out=t["t1"][:, s], in0=t["u1"][:, s], scalar1=-1.0,
                scalar2=1.0, op0=OP.mult, op1=OP.add,
            )
            nc.vector.tensor_scalar(
                out=t["t2"][:, s], in0=t["u1"][:, s], scalar1=1.0,
                scalar2=None, op0=OP.add,
            )
            mm_eng = nc.gpsimd if eng == 'g' else nc.vector
            mm_eng.tensor_mul(t["mm"][:, s], t["t1"][:, s], t["t2"][:, s])
            nc.scalar.activation(t["sr"][:, s], t["mm"][:, s],
                                 AF.Abs_reciprocal_sqrt)

        def chain_ss(k, ws=None):
            w0, w1 = ws if ws else (0, WC)
            s = slice(w0, w1)
            nc.vector.tensor_mul(ssb[:, k, s], chn[k]["t1"][:, s],
                                 chn[k]["sr"][:, s])

        def fillers(n):
            for _ in range(n):
                nc.tensor.matmul(warm[:, 0:256], idw, wsrc[:, 0:256],
                                 start=True, stop=True)

        # ---------------- emission schedule ----------------
        HW_ = WC // 2
        products(0)
        products(1)
        absred(0)
        ps0 = pe_unit(0)
        fillers(6)
        tail(0, ps0)
        products(2)
        ps1 = pe_unit(1)
        fillers(6)
        tail(1, ps1)
        products(3)
        absred(1)
        ps2 = pe_unit(2)
        fillers(6)
        tail(2, ps2)
        products(4)
        ps3 = pe_unit(3)
        fillers(4)
        tail(3, ps3)
        products(5, (0, HW_))
        ps4 = pe_unit(4)
        tail(4, ps4)
        chain(0, eng='g')
        products(5, (HW_, WC))
        absred(2)
        ps5a = pe_unit(5, (0, HW_))
        tail(5, ps5a, (0, HW_))
        chain(1, (0, HW_), eng='g')
        ps5b = pe_unit(5, (HW_, WC))
        tail(5, ps5b, (HW_, WC))
        chain_ss(0)
        chain(1, (HW_, WC), eng='v')
        chain_ss(1, (0, HW_))
        chain_ss(1, (HW_, WC))

        at = outp.tile([P, NCH, WC], F16)
        nc.scalar.activation(
            out=at, in_=ssb, func=AF.Arctan, accum_out=res[:, 7:8]
        )

        nc.sync.dma_start(out=res_out, in_=res)


def _build():
    nc = bacc.Bacc(
        "TRN2", target_bir_lowering=False, debug=False, num_devices=NCORES
    )
    inputs = nc.dram_tensor(
        "inputs", [C, NCH, HC, 2, B, WC], F16, kind="ExternalInput"
    ).ap()
    identf16 = nc.dram_tensor("identf16", [P, P], F16, kind="ExternalInput").ap()
    res_out = nc.dram_tensor("partials", [P, 8], F32, kind="ExternalOutput").ap()
    with tile.TileContext(nc) as tc:
        _body(tc, inputs, identf16, res_out)
    nc.compile()
    return nc


def _get_nc():
    if "nc" not in _CACHE:
        _CACHE["nc"] = _build()
    return _CACHE["nc"]


def _make_in_maps(predictions, targets):
    p = np.asarray(predictions)
    t = np.asarray(targets)
    ident = np.eye(P, dtype=np.float16)
    in_maps = []
    for i in range(NCORES):
        h0 = i * HC
        # [2, B, C, HC, W] -> [C, NCH, HC, 2, B, WC] fp16: one unit (c, k)
        # is a contiguous 1MB block, 8KB per partition-row covering p and t
        both = np.stack([p[:, :, h0 : h0 + HC, :], t[:, :, h0 : h0 + HC, :]])
        arr = np.ascontiguousarray(
            both.reshape(2, B, C, HC, NCH, WC)
            .transpose(2, 4, 3, 0, 1, 5)
            .astype(np.float16)
        )
        in_maps.append({"inputs": arr, "identf16": ident})
    return in_maps


def _combine(results):
    rsum = 0.0
    atsum = 0.0
    for r in results:
        part = np.asarray(r["partials"], dtype=np.float64)
        rsum += part[:, 0:6].sum()
        atsum += part[:, 7].sum()
    loss = rsum / float(B * C * H * W) + atsum
    return np.asarray(np.float32(loss))


def kernel(predictions, targets, _trace=False):
    nc = _get_nc()
    in_maps = _make_in_maps(predictions, targets)
    if _trace:
        out = run_bass_kernel_spmd(
            nc, in_maps, core_ids=list(range(NCORES)), trace=True
        )
        return _combine(out.results), out
    out = run_bass_kernel_spmd(nc, in_maps, core_ids=list(range(NCORES)))
    return _combine(out.results)
